# revision 2
# baseline (speedup 1.0000x reference)
"""DCNv2 (deformable conv v2) Trainium2 kernel — 8-core SPMD, batch x H-half sharding.

v2: phase-reordered for DMA/PE overlap. om conv for both pixel-halves runs
up-front (PE stays hot via warmup/filler trains), index math precedes weight
math so gathers launch early, idx distribution rides the SP queue, gathers
stream continuously, blend consumes rc-split so taps start on half-gathers,
outputs stored f16 and upcast on host.
"""
import numpy as np
from contextlib import ExitStack

import concourse.bass as bass
import concourse.mybir as mybir
import concourse.tile as tile
from concourse import bacc
from concourse.bass_utils import run_bass_kernel_spmd

F16 = np.float16

B, C, H, W = 4, 256, 64, 64
O = 256
K = 3
KK = 9
NCORES = 8
ROWS_PER_CORE = 32
PIX_PER_CORE = ROWS_PER_CORE * W
PH_ROWS = 16
PH_PIX = PH_ROWS * W
XT_ROWS = 65 + H * W + 67  # 4228
PAD66 = 66
P66 = 66

N_WARM = 60     # startup warmup matmuls (128 rows each)
N_FILL = 60     # filler matmuls between om phase and first blend

f16 = mybir.dt.float16
f32 = mybir.dt.float32
i16 = mybir.dt.int16
Alu = mybir.AluOpType
Act = mybir.ActivationFunctionType


def host_prep(x, weight, bias, w_om, b_om):
    """Build all per-core input tensors. x:[B,C,H,W] f32, weight:[O,C,3,3],
    bias:[O], w_om:[27,C,3,3], b_om:[27]."""
    wr = weight.reshape(O, C, KK).transpose(2, 1, 0).reshape(KK * C, O)
    wT_sb = wr.reshape(18, 128, O).transpose(1, 0, 2).reshape(128, 18 * O).astype(F16)
    womr = w_om.reshape(27, C, KK).transpose(2, 1, 0).reshape(KK * C, 27)
    womT_sb = womr.reshape(18, 128, 27).transpose(1, 0, 2).reshape(128, 18 * 27).astype(F16)
    bias_sb = bias.reshape(2, 128).T.astype(np.float32)
    bom_sb = b_om.reshape(27, 1).astype(np.float32)
    ident16 = np.eye(128, dtype=F16)
    # one-hot i16 matrices for the idx wrap transpose: oneh[p, w*128+m] = 1
    # iff p == 16*w + (m % 16)  (replicates the 16-row wrap into all 8 groups)
    oneh = np.zeros((128, 8, 128), dtype=np.float32)
    p = np.arange(128)
    for w in range(8):
        for m in range(128):
            oneh[16 * w + (m % 16), w, m] = 1
    oneh = oneh.reshape(128, 8 * 128)

    per_core = []
    for core in range(NCORES):
        b, h = divmod(core, 2)
        xb = x[b]
        xt = np.zeros((XT_ROWS, C), dtype=F16)
        xt[65:65 + H * W, :] = xb.reshape(C, H * W).T.astype(F16)
        xp = np.zeros((C, PAD66, PAD66), dtype=F16)
        xp[:, 1:65, 1:65] = xb.astype(F16)
        xpc = xp[:, 32 * h:32 * h + 34, :]
        xpad16 = np.ascontiguousarray(xpc).reshape(2, 128, 34 * PAD66)
        part = np.arange(128)
        kk = np.arange(KK)
        ky, kx = kk // K, kk % K
        r2 = part // 64
        ccol = part % 64
        slot = np.arange(8)
        gyk = np.zeros((128, 2, 8, KK), dtype=np.float32)
        for ph in range(2):
            gyk[:, ph, :, :] = ((32 * h + 16 * ph + r2 - 1 + 64)[:, None, None]
                                + 2 * slot[None, :, None] + ky[None, None, :])
        gxk = np.broadcast_to((ccol - 1 + 64)[:, None, None] + kx[None, None, :],
                              (128, 8, KK)).astype(np.float32).copy()
        per_core.append(dict(
            xt=xt, xpad16_0=xpad16[0].copy(), xpad16_1=xpad16[1].copy(),
            wT=wT_sb, womT=womT_sb, bias=bias_sb, bom=bom_sb,
            gyk=gyk.reshape(128, 2 * 8 * KK), gxk=gxk.reshape(128, 8 * KK),
            ident=ident16, oneh=oneh,
        ))
    return per_core


def ap_of(base, offset_delta, dims):
    return bass.AP(tensor=base.tensor, offset=base.offset + offset_delta,
                   ap=[base.ap[0]] + dims)


def build_nc(skip_compile=False, n_warm=N_WARM, n_fill=N_FILL):
    nc = bacc.Bacc("TRN2", target_bir_lowering=False, debug=False, num_devices=8)

    xt_d = nc.dram_tensor("xt", [XT_ROWS, 256], f16, kind="ExternalInput")
    xp_d = [nc.dram_tensor(f"xp{i}", [128, 34, P66], f16, kind="ExternalInput")
            for i in range(2)]
    wT_d = nc.dram_tensor("wT", [128, 18 * 256], f16, kind="ExternalInput")
    womT_d = nc.dram_tensor("womT", [128, 18 * 27], f16, kind="ExternalInput")
    bias_d = nc.dram_tensor("bias", [128, 2], f32, kind="ExternalInput")
    bom_d = nc.dram_tensor("bom", [27, 1], f32, kind="ExternalInput")
    gyk_d = nc.dram_tensor("gyk", [128, 2, 8, KK], f32, kind="ExternalInput")
    gxk_d = nc.dram_tensor("gxk", [128, 8, KK], f32, kind="ExternalInput")
    ident_d = nc.dram_tensor("ident", [128, 128], f16, kind="ExternalInput")
    oneh_d = nc.dram_tensor("oneh", [128, 8 * 128], f32, kind="ExternalInput")
    out_d = nc.dram_tensor("out", [256, 2048], f16, kind="ExternalOutput")

    xt_pairs = bass.AP(tensor=xt_d, offset=0, ap=[[256, XT_ROWS - 1], [1, 512]])

    with tile.TileContext(nc) as tc, ExitStack() as ctx:
        const = ctx.enter_context(tc.tile_pool(name="const", bufs=1))
        xp = [const.tile([128, 34, P66], f16, name=f"xp{i}", tag=f"xp{i}") for i in range(2)]
        wT = const.tile([128, 18 * 256], f16, name="wT", tag="wT")
        womT = const.tile([128, 18 * 27], f16, name="womT", tag="womT")
        bias_t = const.tile([128, 2], f32, name="bias", tag="bias")
        bom_t = const.tile([27, 1], f32, name="bom", tag="bom")
        gyk_t = const.tile([128, 2, 8, KK], f32, name="gyk", tag="gyk")
        gxk_t = const.tile([128, 8, KK], f32, name="gxk", tag="gxk")
        ident_t = const.tile([128, 128], f16, name="ident", tag="ident")
        oneh_t = const.tile([128, 8 * 128], f32, name="oneh", tag="oneh")
        # load order matters: om-conv deps first, then main-GEMM deps.
        # xp rows [0:19] are enough for the ph0 om conv — load those first so
        # the om GEMM starts ~1.5us earlier; rows [19:34] follow.
        nc.sync.dma_start(out=womT[:], in_=womT_d.ap())
        for i in range(2):
            nc.sync.dma_start(out=xp[i][:, 0:19], in_=xp_d[i].ap()[:, 0:19])
        nc.sync.dma_start(out=bom_t[:], in_=bom_d.ap())
        for i in range(2):
            nc.sync.dma_start(out=xp[i][:, 19:34], in_=xp_d[i].ap()[:, 19:34])
        for t_, d_ in ((gyk_t, gyk_d), (gxk_t, gxk_d),
                       (ident_t, ident_d), (oneh_t, oneh_d), (wT, wT_d),
                       (bias_t, bias_d)):
            nc.sync.dma_start(out=t_[:], in_=d_.ap())

        # psum pools: om ring(2) + colp(2, also hosts the omT psum) + outp(4) = 8 banks
        omp_pool = ctx.enter_context(tc.tile_pool(name="omp", bufs=2, space="PSUM"))
        colp_pool = ctx.enter_context(tc.tile_pool(name="colp", bufs=2, space="PSUM"))
        outp_pool = ctx.enter_context(tc.tile_pool(name="outp", bufs=1, space="PSUM"))

        omsb_pool = ctx.enter_context(tc.tile_pool(name="omsb", bufs=2))
        math_pool = ctx.enter_context(tc.tile_pool(name="math", bufs=2))
        idxw_pool = ctx.enter_context(tc.tile_pool(name="idxw", bufs=2))
        g_pool = ctx.enter_context(tc.tile_pool(name="g", bufs=6))
        diag_pool = ctx.enter_context(tc.tile_pool(name="diag", bufs=64))
        cols_pool = ctx.enter_context(tc.tile_pool(name="cols", bufs=6))
        outs_pool = ctx.enter_context(tc.tile_pool(name="outs", bufs=2))

        # PE warm-up train: ramp the clock gate before the om GEMM.
        warm_src = const.tile([128, 128], f16, name="warm_src", tag="warm_src")
        nc.gpsimd.memset(warm_src[:], 0.0)

        def emit_warm(n):
            warm_ps = omp_pool.tile([128, 512], f32, name="warm_ps", tag="psum_om")
            for wi in range(n):
                nc.tensor.matmul(warm_ps[:, (wi % 4) * 128:(wi % 4) * 128 + 128],
                                 lhsT=warm_src[:], rhs=warm_src[:],
                                 start=True, stop=True)

        emit_warm(n_warm)

        # ---------- om conv GEMM + transpose ----------
        def emit_om_block(ph, n5, om_sb):
            psum_om = omp_pool.tile([27, 512], f32, name="psum_om", tag="psum_om")
            for t in range(18):
                k, chh = divmod(t, 2)
                ky, kx = k // 3, k % 3
                r0 = 16 * ph + ky + n5 * 8
                rhs = xp[chh][:, r0:r0 + 8, kx:kx + 64]
                nc.tensor.matmul(
                    psum_om[:], lhsT=womT[:, t * 27:(t + 1) * 27], rhs=rhs,
                    start=(t == 0), stop=(t == 17))
            nc.scalar.activation(om_sb[:, n5 * 512:(n5 + 1) * 512], psum_om[:],
                                 Act.Identity, bias=bom_t[:])

        def emit_omT(ph, om_sb):
            ompm = math_pool.tile([128, 8, 27], f32, name="ompm", tag="ompm")
            pom = colp_pool.tile([128, 8, 27], f32, name="pom", tag="pc",
                                 padded_shape=[128, 8, 64])
            for q in range(8):
                nc.tensor.matmul(pom[:, q], lhsT=om_sb[:, q * 128:(q + 1) * 128],
                                 rhs=ident_t[0:27, 0:27], start=True, stop=True)
            nc.scalar.activation(ompm[:], pom[:], Act.Copy)
            return ompm

        V = nc.vector

        def mt(tag):
            return math_pool.tile([128, 8, KK], f32, name=tag, tag=tag)

        # ---------- index math (gather-critical subset) ----------
        def emit_idx_math(ph, ompm):
            s = {}
            dy = ompm[:, :, 0:KK]
            dx = ompm[:, :, KK:2 * KK]
            ty, tx_ = mt("ty"), mt("tx")
            y064, x064 = mt("y064"), mt("x064")
            ya, xa, idxf, idxf0, idxf1 = mt("ya"), mt("xa"), mt("idxf"), mt("if0"), mt("if1")
            yi32 = math_pool.tile([128, 8, KK], mybir.dt.int32, name="yi32", tag="yi32")
            xi32 = math_pool.tile([128, 8, KK], mybir.dt.int32, name="xi32", tag="xi32")
            yif, xif = mt("yif"), mt("xif")
            gq, gqx = mt("gq"), mt("gqx")
            V.tensor_tensor(ty[:], dy, gyk_t[:, ph], Alu.add)
            V.tensor_tensor(tx_[:], dx, gxk_t[:], Alu.add)
            V.tensor_copy(yi32[:], ty[:])
            V.tensor_copy(yif[:], yi32[:])
            V.tensor_tensor(gq[:], yif[:], ty[:], Alu.is_gt)
            V.tensor_tensor(y064[:], yif[:], gq[:], Alu.subtract)
            V.tensor_copy(xi32[:], tx_[:])
            V.tensor_copy(xif[:], xi32[:])
            V.tensor_tensor(gqx[:], xif[:], tx_[:], Alu.is_gt)
            V.tensor_tensor(x064[:], xif[:], gqx[:], Alu.subtract)
            V.tensor_scalar(ya[:], y064[:], 63.0, 127.0, Alu.max, Alu.min)
            V.tensor_scalar(xa[:], x064[:], 63.0, 127.0, Alu.max, Alu.min)
            V.scalar_tensor_tensor(idxf[:], ya[:], 64.0, xa[:], Alu.mult, Alu.add)
            V.tensor_scalar_add(idxf0[:], idxf[:], -4095.0)
            V.tensor_scalar_add(idxf1[:], idxf[:], -4031.0)
            idx_pm = math_pool.tile([128, KK, 2, 8], f32, name="idx_pm", tag="idx_pm")
            ipb = idx_pm[:]
            cast0 = bass.AP(tensor=ipb.tensor, offset=ipb.offset,
                            ap=[ipb.ap[0], [1, 8], [16, KK]])
            cast1 = bass.AP(tensor=ipb.tensor, offset=ipb.offset + 8,
                            ap=[ipb.ap[0], [1, 8], [16, KK]])
            V.tensor_copy(cast0, idxf0[:])
            V.tensor_copy(cast1, idxf1[:])
            s.update(ty=ty, tx=tx_, y064=y064, x064=x064, idx_pm=idx_pm, ompm=ompm)
            return s

        # ---------- idx wrap via one-hot PE transpose ----------
        # idxw[16g+r, (k,rc,q)*8 + w] = idx_pm[16w + r, (k,rc,q)] for all g.
        # 4 chunks x 8 one-hot matmuls (36 rows each) into int32 psum, then
        # DVE copies psum -> i16 idxw. No DMA involved.
        def emit_dist(ph, s):
            idx_pm = s["idx_pm"]
            idxw = idxw_pool.tile([128, KK, 2, 64], i16, name="idxw", tag="idxw")
            ipb = idx_pm[:]
            iwb = idxw[:]
            for c in range(4):
                pd = omp_pool.tile([128, 288], f32, name="pdist",
                                   tag="psum_om")
                for w in range(8):
                    rhs = bass.AP(tensor=ipb.tensor, offset=ipb.offset + 36 * c,
                                  ap=[ipb.ap[0], [1, 36]])
                    dst = bass.AP(tensor=pd.tensor, offset=pd.offset + w,
                                  ap=[pd.ap[0], [8, 36]])
                    nc.tensor.matmul(dst, lhsT=oneh_t[:, w * 128:(w + 1) * 128],
                                     rhs=rhs, start=True, stop=True)
                dstw = bass.AP(tensor=iwb.tensor, offset=iwb.offset + 288 * c,
                               ap=[iwb.ap[0], [1, 288]])
                V.tensor_copy(dstw, pd[:])
            s["idxw"] = idxw

        # ---------- bilinear weight math ----------
        def emit_weight_math(ph, s):
            ty, tx_, y064, x064 = s["ty"], s["tx"], s["y064"], s["x064"]
            ml = s["ompm"][:, :, 2 * KK:3 * KK]
            fry, frx, m_t = mt("fry"), mt("frx"), mt("m")
            s0y, g0, ay0, ay0m = mt("s0y"), mt("g0"), mt("ay0"), mt("ay0m")
            g1, ay1, ay1m = mt("g1"), mt("ay1"), mt("ay1m")
            s0x, g0x, bx0, g1x, bx1 = mt("s0x"), mt("g0x"), mt("bx0"), mt("g1x"), mt("bx1")
            wc = [mt(f"wc{i}") for i in range(4)]
            V.tensor_tensor(fry[:], ty[:], y064[:], Alu.subtract)
            V.tensor_tensor(frx[:], tx_[:], x064[:], Alu.subtract)
            nc.scalar.activation(m_t[:], ml, Act.Sigmoid)
            V.tensor_scalar(s0y[:], fry[:], -1.0, 1.0, Alu.mult, Alu.add)
            V.scalar_tensor_tensor(g0[:], y064[:], 64.0, s0y[:], Alu.is_ge, Alu.mult)
            V.scalar_tensor_tensor(ay0[:], y064[:], 127.0, g0[:], Alu.is_le, Alu.mult)
            V.tensor_tensor(ay0m[:], ay0[:], m_t[:], Alu.mult)
            V.scalar_tensor_tensor(g1[:], y064[:], 63.0, fry[:], Alu.is_ge, Alu.mult)
            V.scalar_tensor_tensor(ay1[:], y064[:], 126.0, g1[:], Alu.is_le, Alu.mult)
            V.tensor_tensor(ay1m[:], ay1[:], m_t[:], Alu.mult)
            V.tensor_scalar(s0x[:], frx[:], -1.0, 1.0, Alu.mult, Alu.add)
            V.scalar_tensor_tensor(g0x[:], x064[:], 64.0, s0x[:], Alu.is_ge, Alu.mult)
            V.scalar_tensor_tensor(bx0[:], x064[:], 127.0, g0x[:], Alu.is_le, Alu.mult)
            V.scalar_tensor_tensor(g1x[:], x064[:], 63.0, frx[:], Alu.is_ge, Alu.mult)
            V.scalar_tensor_tensor(bx1[:], x064[:], 126.0, g1x[:], Alu.is_le, Alu.mult)
            V.tensor_tensor(wc[0][:], ay0m[:], bx0[:], Alu.mult)
            V.tensor_tensor(wc[1][:], ay0m[:], bx1[:], Alu.mult)
            V.tensor_tensor(wc[2][:], ay1m[:], bx0[:], Alu.mult)
            V.tensor_tensor(wc[3][:], ay1m[:], bx1[:], Alu.mult)
            s["wc"] = wc

        def emit_diags(ph, s, k):
            wc = s["wc"]
            diags = []
            for q in range(8):
                d4 = []
                for c4 in range(4):
                    diag = diag_pool.tile([128, 128], f16, name="diag", tag="diag")
                    wsl = wc[c4][:, q, k:k + 1]
                    nc.vector.tensor_scalar(diag[:], ident_t[:], wsl, None, Alu.mult)
                    d4.append(diag)
                diags.append(d4)
            return diags

        # ---------- emit phases ----------
        st = [None, None]
        om_sb0 = omsb_pool.tile([27, PH_PIX], f16, name="om_sb", tag="om_sb")
        om_sb1 = omsb_pool.tile([27, PH_PIX], f16, name="om_sb", tag="om_sb")

        emit_om_block(0, 0, om_sb0)
        emit_om_block(0, 1, om_sb0)
        ompm0 = emit_omT(0, om_sb0)
        emit_om_block(1, 0, om_sb1)     # PE busy while DVE runs ph0 idx math
        st[0] = emit_idx_math(0, ompm0)
        emit_dist(0, st[0])             # PE one-hot transposes + DVE copies
        emit_om_block(1, 1, om_sb1)
        ompm1 = emit_omT(1, om_sb1)
        emit_weight_math(0, st[0])
        diag_cache = {}
        diag_cache[(0, 0)] = emit_diags(0, st[0], 0)
        diag_cache[(0, 1)] = emit_diags(0, st[0], 1)
        st[1] = emit_idx_math(1, ompm1)

        # ---------- gathers ph0 on Pool ----------
        G_tiles = {}

        def emit_half_gather(ph, k, rc, hh):
            Gt = g_pool.tile([128, 4, 512], f16, name="Gh", tag="Gh", bufs=4)
            nc.gpsimd.dma_gather(
                Gt[:], xt_pairs,
                st[ph]["idxw"][:, k, rc, hh * 32:(hh + 1) * 32],
                PH_PIX // 2, PH_PIX // 2,
                elem_size=512, elem_step=256, queue_num=0)
            G_tiles[(ph, k, rc, hh)] = Gt

        def emit_gather(ph, k, rc):
            if ph == 1 and k == KK - 1:
                return  # emitted split, qh-major, by the caller
            Gt = g_pool.tile([128, 8, 512], f16, name=f"G{rc}", tag=f"G{rc}")
            nc.gpsimd.dma_gather(
                Gt[:], xt_pairs, st[ph]["idxw"][:, k, rc, :], PH_PIX, PH_PIX,
                elem_size=512, elem_step=256, queue_num=0)
            G_tiles[(ph, k, rc)] = Gt

        def g_slice(ph, k, rc, q, sl):
            if (ph, k, rc) in G_tiles:
                return G_tiles[(ph, k, rc)][:, q, sl]
            return G_tiles[(ph, k, rc, q // 4)][:, q % 4, sl]

        for k in range(KK):
            for rc in range(2):
                emit_gather(0, k, rc)

        # ---------- PE filler between om phase and first blend ----------
        emit_warm(n_fill)

        # ---------- blend + main GEMM per ph ----------
        osb_cache = {}

        def emit_osb(ph, psum_out, n5):
            # bias + f16 cast + store for one pixel-half; o2=0 on Act, o2=1 on
            # DVE so the two halves drain in parallel
            od = out_d.ap()
            for o2 in range(2):
                if (ph, o2) not in osb_cache:
                    osb_cache[(ph, o2)] = outs_pool.tile(
                        [128, PH_PIX], f16, name=f"osb{o2}", tag=f"osb{o2}",
                        bufs=2)
                osb = osb_cache[(ph, o2)]
                src = psum_out[o2][:, n5 * 512:(n5 + 1) * 512]
                dsl = osb[:, n5 * 512:(n5 + 1) * 512]
                if o2 == 0:
                    nc.scalar.activation(dsl, src, Act.Identity,
                                         bias=bias_t[:, o2:o2 + 1])
                else:
                    V.tensor_scalar(dsl, src, bias_t[:, o2:o2 + 1], None,
                                    Alu.add)
                dst = bass.AP(tensor=od.tensor,
                              offset=(od.offset + o2 * 128 * 2048
                                      + ph * PH_PIX + n5 * 512),
                              ap=[[2048, 128], [1, 512]])
                nc.sync.dma_start(out=dst, in_=dsl)

        for ph in range(2):
            psum_out = [outp_pool.tile([128, PH_PIX], f32, name=f"po{o2}", tag=f"po{o2}")
                        for o2 in range(2)]
            for k in range(KK):
                if (ph, k) in diag_cache:
                    diags = diag_cache[(ph, k)]
                else:
                    diags = emit_diags(ph, st[ph], k)
                # qh-major: blend both chh for a pixel-half, then that half's
                # main-GEMM step (n5 == qh) — hides cols-act latency and
                # minimizes post-last-gather work on the final tap.
                cols_t = [cols_pool.tile([128, PH_PIX], f16, name="cols",
                                         tag="cols") for _ in range(2)]
                for qh in range(2):
                    for chh in range(2):
                        pc = colp_pool.tile([128, 512], f32, name="pc", tag="pc")
                        # each qq-block's 4-corner accumulation group must be
                        # contiguous: interleaved start/stop groups on one
                        # psum bank corrupt the accumulation on HW
                        for qq in range(4):
                            q = qh * 4 + qq
                            for c4 in range(4):
                                rc, xc = divmod(c4, 2)
                                sl = slice(xc * 256 + chh * 128,
                                           xc * 256 + chh * 128 + 128)
                                nc.tensor.matmul(
                                    pc[:, qq * 128:(qq + 1) * 128],
                                    lhsT=g_slice(ph, k, rc, q, sl),
                                    rhs=diags[q][c4][:],
                                    start=(c4 == 0), stop=(c4 == 3))
                        nc.scalar.activation(cols_t[chh][:, qh * 512:(qh + 1) * 512],
                                             pc[:], Act.Copy)
                    for chh in range(2):
                        t = k * 2 + chh
                        for o2 in range(2):
                            nc.tensor.matmul(
                                psum_out[o2][:, qh * 512:(qh + 1) * 512],
                                lhsT=wT[:, t * 256 + o2 * 128:
                                        t * 256 + o2 * 128 + 128],
                                rhs=cols_t[chh][:, qh * 512:(qh + 1) * 512],
                                start=(t == 0), stop=(t == 17))
                    if k == KK - 1:
                        # this pixel-half's accumulation is complete — drain it
                        emit_osb(ph, psum_out, qh)
                if ph == 0 and k == 0:
                    # ph1 idx wrap: PE is otherwise idle-ish here; DVE math1
                    # is ready by now
                    emit_dist(1, st[1])
                    emit_weight_math(1, st[1])
                    for kk_ in range(KK):
                        for rc in range(2):
                            emit_gather(1, kk_, rc)
                    # final tap gathered in qh-major halves to shrink the tail
                    for hh in range(2):
                        for rc in range(2):
                            emit_half_gather(1, KK - 1, rc, hh)

            # osb emission happens inside the k-loop at the last tap

    if not skip_compile:
        nc.compile()
    return nc


_NC_CACHE = {}


def _get_nc():
    if "nc" not in _NC_CACHE:
        _NC_CACHE["nc"] = build_nc()
    return _NC_CACHE["nc"]


def kernel(x, weight, bias, w_om, b_om):
    x = np.ascontiguousarray(np.asarray(x, dtype=np.float32))
    weight = np.asarray(weight, dtype=np.float32)
    bias = np.asarray(bias, dtype=np.float32)
    w_om = np.asarray(w_om, dtype=np.float32)
    b_om = np.asarray(b_om, dtype=np.float32)

    per_core = host_prep(x, weight, bias, w_om, b_om)
    in_maps = []
    for pc in per_core:
        in_maps.append({
            "xt": pc["xt"],
            "xp0": pc["xpad16_0"].reshape(128, 34, 66),
            "xp1": pc["xpad16_1"].reshape(128, 34, 66),
            "wT": pc["wT"], "womT": pc["womT"],
            "bias": pc["bias"], "bom": pc["bom"],
            "gyk": pc["gyk"].reshape(128, 2, 8, 9),
            "gxk": pc["gxk"].reshape(128, 8, 9),
            "ident": pc["ident"], "oneh": pc["oneh"],
        })

    nc = _get_nc()
    res = run_bass_kernel_spmd(nc, in_maps, list(range(NCORES)))

    out = np.zeros((B, O, H, W), dtype=np.float32)
    for core in range(NCORES):
        b, h = divmod(core, 2)
        oc = res.results[core]["out"].astype(np.float32)
        out[b, :, 32 * h:32 * h + 32, :] = oc.reshape(O, ROWS_PER_CORE, W)
    return out
